# revision 2
# baseline (speedup 1.0000x reference)
"""Trainium2 Bass kernel v2 for nn_BuildVolume.

Same math as v1 (dense-hat bilinear via TensorE x-contraction + per-sample
y-hat mult/reduce + gated 1x1 conv) restructured for engine balance:

- vv chunks grouped by |y-offset| in pairs: windows {44,32,24,12} (vs 39/29/13
  shared across 3 rows) and per-vv y-shifts baked into the slab by the host.
- x-hats built once per h-block (8 h) and cached, not rebuilt per chunk.
- stage-2 split across engines: Pool (gpsimd) multiplies straight out of
  PSUM (fused f32 read + bf16 cast), ACT casts for DVE bf16 multiplies,
  DVE runs a 2-level pairwise tree + short tensor_reduce instead of a full
  1x-rate reduce; gate applied in one pass per (chunk,h).
- conv transposes on the PE (identity matmul) instead of DMA transpose.

Sharding: H split 8 x 16, no collectives.
"""

import numpy as np
import ml_dtypes

import concourse.bacc as bacc
import concourse.mybir as mybir
from concourse.tile import TileContext
from concourse.masks import make_identity
from concourse import bass_utils

F32 = mybir.dt.float32
BF16 = mybir.dt.bfloat16
AX = mybir.AxisListType
OP = mybir.AluOpType
ACTF = mybir.ActivationFunctionType

B, H, W, N, C, OUT, M = 1, 128, 128, 4, 8, 64, 9
V = M * M
NCORE = 8
HPC = 16                     # rows per core
HB = 8                       # rows per h-block
PADX = 21
XP = 74
CH_VVS = [[0, 8], [1, 7], [2, 6], [3, 5, 4]]
WINS = [44, 32, 24, 16]
SHIFTS = [21, 16, 12, 7]
YCMAX = WINS[0] + HB - 1     # 51 (uniform slab tile y extent)
NCH = len(CH_VVS)
KCH = 256                    # padded k per chunk (2 x 128)
SUMW = sum(WINS)             # 112

# engine assignment per ui: mult path 'A' (ACT cast + DVE mult),
# 'P' (gpsimd direct from PSUM), 'D' (DVE direct from PSUM)
MULT_PATH = list('QQQQQQQDD')
L1_ENG = list('DDDDDDDDD')
L2_ENG = list('DDDDDDDDD')
DMA_SPLIT = 4

KOFF = [0]
for _vvs in CH_VVS:
    KOFF.append(KOFF[-1] + len(_vvs) * M * C)


def _bf16(a):
    return np.asarray(a, dtype=ml_dtypes.bfloat16)


def prep_core(k, deltmap, imageMxM, x_g, conv_w, conv_b):
    """Host-side shard prep for core k (layout only)."""
    d = deltmap[0]
    g = x_g[0, 0]
    hg0 = HPC * k
    imgs = np.moveaxis(imageMxM[0], (3, 4), (0, 1)).reshape(V, H, W, C)
    s128 = np.arange(128)
    p = np.arange(XP)

    # slabs: per chunk one dram tensor [2*nvv, XP, M*4*YCMAX*C] bf16
    slabs = {}
    for ch, vvs in enumerate(CH_VVS):
        sh = SHIFTS[ch]
        nvv = len(vvs)
        sl = np.zeros((2 * nvv, XP, M, 4, YCMAX, C), np.float32)
        for hb in range(2):
            hb0 = hg0 + hb * HB
            y0 = hb0 - sh
            ys, ye = max(0, y0), min(H, y0 + YCMAX)
            for vi, vv in enumerate(vvs):
                for ui in range(M):
                    im = imgs[vv * M + ui]
                    for wb in range(4):
                        x0 = wb * 32 - PADX
                        xs, xe = max(0, x0), min(W, x0 + XP)
                        sl[hb * nvv + vi, xs - x0:xe - x0, ui, wb,
                           ys - y0:ye - y0, :] = \
                            im[ys:ye, xs:xe, :].transpose(1, 0, 2)
        slabs[f"slab{ch}"] = _bf16(sl.reshape(2 * nvv, XP, -1))

    # tb: [HPC, XP, 4*128] f32
    tb = np.zeros((HPC, XP, 4, 128), np.float32)
    for h in range(HPC):
        for wb in range(4):
            tb[h, :, wb, :] = d[hg0 + h, wb * 32 + s128 // 4, s128 % 4][None, :]

    # awx: [XP, 4*128] f32
    awx = np.zeros((XP, 4, 128), np.float32)
    for wb in range(4):
        wglob = wb * 32 + s128 // 4
        aw = wglob * (W / (W - 1.0)) - 0.5 - (wb * 32 - PADX)
        awx[:, wb, :] = aw[None, :] - p[:, None]

    # dcol: [128, HPC*4] f32
    dcol = np.zeros((128, HPC, 4), np.float32)
    for h in range(HPC):
        for wb in range(4):
            dcol[:, h, wb] = d[hg0 + h, wb * 32 + s128 // 4, s128 % 4]

    # yio: [128, SUMW] f32, h-independent (per-h frac offset folded into m0)
    yio = np.zeros((SUMW,), np.float32)
    off = 0
    for ch in range(NCH):
        win, sh = WINS[ch], SHIFTS[ch]
        yio[off:off + win] = np.arange(win) - sh + 0.5
        off += win
    yio = np.ascontiguousarray(np.broadcast_to(
        yio.reshape(1, SUMW), (128, SUMW)))

    # xg: [HPC, 128, 4*81] bf16, global k-order (ch, vi, ui)
    vord = [vv * M + uu for vvs in CH_VVS for vv in vvs for uu in range(M)]
    xg = np.zeros((HPC, 128, 4, V), np.float32)
    for h in range(HPC):
        for wb in range(4):
            xg[h, :, wb, :] = g[hg0 + h, wb * 32 + s128 // 4][:, vord]
    xg = _bf16(xg.reshape(HPC, 128, 4 * V))

    # wt: [128, NCH*2*OUT] bf16
    wt = np.zeros((128, NCH, 2, OUT), np.float32)
    for ch, vvs in enumerate(CH_VVS):
        nvv = len(vvs)
        for j in range(2):
            for pp in range(128):
                kk = j * 128 + pp
                if kk < nvv * M * C:
                    vi, r = divmod(kk, M * C)
                    uu, c = divmod(r, C)
                    wt[pp, ch, j, :] = conv_w[:, c * V + vvs[vi] * M + uu]
    wt = _bf16(wt.reshape(128, NCH * 2 * OUT))

    bias = conv_b.reshape(OUT, 1).astype(np.float32)

    hfrac = np.ascontiguousarray(np.broadcast_to(
        ((hg0 + np.arange(HPC)) / (H - 1.0)).astype(np.float32)
        .reshape(1, HPC), (128, HPC)))

    out = dict(tb=tb.reshape(HPC, XP, 512), awx=awx.reshape(XP, 512),
               dcol=dcol.reshape(128, HPC * 4), yio=yio, xg=xg, wt=wt,
               bias=bias, hfrac=hfrac)
    out.update(slabs)
    return out


def _emit_hx(nc, hxp, tbp, scr, tb_d, awxT, hb):
    """x-hats for one h-block: [XP, HB*M*4*128] bf16."""
    hx = hxp.tile([XP, HB * M * 512], BF16, name="hx", tag="hx")
    hx5 = hx[:].rearrange("p (h u s) -> p h u s", h=HB, u=M)
    for hl in range(HB):
        h = hb * HB + hl
        tbT = tbp.tile([XP, 512], F32, name="tbT", tag="tb")
        nc.sync.dma_start(tbT[:], tb_d[h])
        for ui in range(M):
            ic = float(M // 2 - ui)
            u = scr.tile([XP, 512], F32, name="u", tag="u")
            nc.vector.scalar_tensor_tensor(
                u[:], tbT[:], ic, awxT[:], OP.mult, OP.add)
            nc.scalar.activation(u[:], u[:], ACTF.Abs)
            nc.scalar.activation(hx5[:, hl, ui, :], u[:],
                                 ACTF.Relu, bias=1.0, scale=-1.0)
    return hx5


def _emit_yhats(nc, hyp, scr, dcolT, yioT, hfracT, ch, h, yoff):
    vvs = CH_VVS[ch]
    nvv = len(vvs)
    win = WINS[ch]
    hy = hyp.tile([128, 3 * 4 * WINS[0]], BF16, name="hy", tag="hy")
    hy4 = hy[:, :nvv * 4 * win].rearrange("p (v wb y) -> p v wb y",
                                          v=nvv, wb=4)
    for vi, vv in enumerate(vvs):
        bcoef = float(M // 2 - vv)
        m0 = scr.tile([128, 4], F32, name="m0", tag="m0")
        nc.vector.scalar_tensor_tensor(
            m0[:], dcolT[:, h * 4:(h + 1) * 4], bcoef,
            hfracT[:, h:h + 1].broadcast_to((128, 4)), OP.mult, OP.add)
        u4 = scr.tile([128, 4 * WINS[0]], F32, name="u4", tag="u4")
        u4v = u4[:, :4 * win].rearrange("p (wb y) -> p wb y", wb=4)
        ys = yoff[ch]
        nc.vector.tensor_tensor(
            u4v,
            yioT[:, ys:ys + win].unsqueeze(1).broadcast_to((128, 4, win)),
            m0[:].unsqueeze(2).broadcast_to((128, 4, win)),
            OP.subtract)
        nc.scalar.activation(u4[:, :4 * win], u4[:, :4 * win], ACTF.Abs)
        nc.scalar.activation(
            hy4[:, vi],
            u4[:, :4 * win].rearrange("p (wb y) -> p wb y", wb=4),
            ACTF.Relu, bias=1.0, scale=-1.0)
    return hy4


def _emit_stage12(nc, pools, sltiles, hx5, hy4, c3, ch, hl, ui):
    """stage-1 matmuls + stage-2 mult/tree/reduce for one (ch, hl, ui)."""
    tpsp, m5p, mbp, l1p, l2p = pools
    vvs = CH_VVS[ch]
    nvv = len(vvs)
    win = WINS[ch]
    nk = nvv * M * C
    cw = C * win
    nf = nvv * cw
    soff = 512 if nf > 512 else cw      # per-vi slot stride (bank-safe)
    lhs = hx5[:, hl, ui, :].rearrange("p (wb s) -> p wb s", wb=4)
    mp = MULT_PATH[ui]
    for wbp in range(2):
        m5 = m5p.tile([128, 2 * 704], BF16, name="m5", tag="m5")
        for wbi in range(2):
            wb = wbp * 2 + wbi
            tps = tpsp.tile([128, 1024], F32, name="tps", tag="tps")
            for vi in range(nvv):
                rhs = sltiles[vi][:, ui, wb, hl:hl + win, :] \
                    .rearrange("p y c -> p c y")
                nc.tensor.matmul(
                    tps[:, vi * soff:vi * soff + cw],
                    lhs[:, wb, :], rhs, start=True, stop=True)
            tview = tps[:, :nvv * soff].rearrange(
                "p (v r) -> p v r", v=nvv)[:, :, :cw].rearrange(
                "p v (c y) -> p v c y", c=C)
            hyv = hy4[:, :, wb, :].unsqueeze(2) \
                .broadcast_to((128, nvv, C, win))
            m5h = m5[:, wbi * nf:(wbi + 1) * nf].rearrange(
                "p (v c y) -> p v c y", v=nvv, c=C)
            if mp in ('A', 'Q'):
                mb = mbp.tile([128, 704], BF16, name="mb", tag="mbf")
                nc.scalar.copy(
                    mb[:, :nf].rearrange("p (v c y) -> p v c y",
                                         v=nvv, c=C), tview)
                eng = nc.vector if mp == 'A' else nc.gpsimd
                eng.tensor_tensor(
                    m5h, mb[:, :nf].rearrange("p (v c y) -> p v c y",
                                              v=nvv, c=C),
                    hyv, OP.mult)
            else:
                nc.vector.tensor_tensor(m5h, tview, hyv, OP.mult)
        # tree reduce: win -> win/2 -> win/4 -> 1
        g2 = 2 * nvv * C
        hw = win // 2
        qw = win // 4
        m3 = m5[:, :2 * nf].rearrange("p (g y) -> p g y", g=g2)
        l1 = l1p.tile([128, 2 * 352], BF16, name="l1", tag="l1")
        l1v = l1[:, :g2 * hw].rearrange("p (g y) -> p g y", g=g2)
        eng1 = nc.vector if L1_ENG[ui] == 'D' else nc.gpsimd
        eng1.tensor_tensor(l1v, m3[:, :, 0:hw], m3[:, :, hw:win], OP.add)
        l2 = l2p.tile([128, 2 * 176], BF16, name="l2", tag="l2")
        l2v = l2[:, :g2 * qw].rearrange("p (g y) -> p g y", g=g2)
        eng2 = nc.vector if L2_ENG[ui] == 'D' else nc.gpsimd
        eng2.tensor_tensor(l2v, l1v[:, :, 0:qw], l1v[:, :, qw:hw], OP.add)
        red_out = c3[:, wbp * 2:(wbp + 1) * 2, :nk].rearrange(
            "p w (v u c) -> p w v u c", v=nvv, u=M)[:, :, :, ui, :]
        # <=11 terms, almost all zero (hat support is 2 taps): bf16 safe
        with nc.allow_low_precision(reason="short window sum, 2 live taps"):
            nc.vector.tensor_reduce(
                red_out, l2v.rearrange("p (w v c) y -> p w v c y",
                                       w=2, v=nvv),
                AX.X, OP.add)


def _emit_conv(nc, pools, c3, wtT, ident, partials, ch, h):
    trp, cvp, ctp = pools
    for wb in range(4):
        cvt = cvp.tile([OUT, 128], F32, name="cvt", tag="cvt")
        for j in range(2):
            tr = trp.tile([128, 128], BF16, name="tr", tag="tr")
            nc.tensor.transpose(tr[:], c3[:, wb, j * 128:(j + 1) * 128],
                                ident[:])
            ct = ctp.tile([128, 128], BF16, name="ct", tag="ct")
            nc.vector.tensor_copy(ct[:], tr[:])
            nc.tensor.matmul(
                cvt[:],
                wtT[:, (ch * 2 + j) * OUT:(ch * 2 + j + 1) * OUT],
                ct[:], start=(j == 0), stop=(j == 1))
        ps = partials[(wb % 2) * OUT:(wb % 2 + 1) * OUT,
                      (h * 2 + wb // 2) * 128:(h * 2 + wb // 2 + 1) * 128]
        if ch == 0:
            nc.vector.tensor_copy(ps, cvt[:])
        else:
            nc.vector.tensor_tensor(ps, ps, cvt[:], OP.add)


def build_nc(reps=1):
    nc = bacc.Bacc("TRN2", target_bir_lowering=False)

    slab_d = []
    for ch, vvs in enumerate(CH_VVS):
        slab_d.append(nc.dram_tensor(
            f"slab{ch}", [2 * len(vvs), XP, M * 4 * YCMAX * C], BF16,
            kind="ExternalInput"))
    tb_d = nc.dram_tensor("tb", [HPC, XP, 512], F32, kind="ExternalInput")
    awx_d = nc.dram_tensor("awx", [XP, 512], F32, kind="ExternalInput")
    dcol_d = nc.dram_tensor("dcol", [128, HPC * 4], F32, kind="ExternalInput")
    yio_d = nc.dram_tensor("yio", [128, SUMW], F32, kind="ExternalInput")
    xg_d = nc.dram_tensor("xg", [HPC, 128, 4 * V], BF16, kind="ExternalInput")
    wt_d = nc.dram_tensor("wt", [128, NCH * 2 * OUT], BF16,
                          kind="ExternalInput")
    bias_d = nc.dram_tensor("bias", [OUT, 1], F32, kind="ExternalInput")
    hfrac_d = nc.dram_tensor("hfrac", [128, HPC], F32, kind="ExternalInput")
    out_d = nc.dram_tensor("out", [OUT, HPC, W, N], F32, kind="ExternalOutput")

    dmaq = [nc.sync, nc.scalar, nc.gpsimd]
    yoff = [0]
    for ch in range(NCH):
        yoff.append(yoff[-1] + WINS[ch])

    with TileContext(nc) as tc:
        with (
            tc.tile_pool(name="consts", bufs=1) as cp,
            tc.tile_pool(name="hxp", bufs=1) as hxp,
            tc.tile_pool(name="slabp", bufs=3) as sp,
            tc.tile_pool(name="tbp", bufs=2) as tbp,
            tc.tile_pool(name="scr", bufs=2) as scr,
            tc.tile_pool(name="hyp", bufs=2) as hyp,
            tc.tile_pool(name="m5p", bufs=3) as m5p,
            tc.tile_pool(name="mbp", bufs=2) as mbp,
            tc.tile_pool(name="l1p", bufs=2) as l1p,
            tc.tile_pool(name="l2p", bufs=2) as l2p,
            tc.tile_pool(name="costp", bufs=2) as costp,
            tc.tile_pool(name="xgp", bufs=2) as xgp,
            tc.tile_pool(name="ctp", bufs=2) as ctp,
            tc.tile_pool(name="outp", bufs=2) as outp,
            tc.tile_pool(name="tpsp", bufs=2, space="PSUM") as tpsp,
            tc.tile_pool(name="trp", bufs=1, space="PSUM") as trp,
            tc.tile_pool(name="cvp", bufs=1, space="PSUM") as cvp,
        ):
            awxT = cp.tile([XP, 512], F32)
            nc.sync.dma_start(awxT[:], awx_d[:])
            dcolT = cp.tile([128, HPC * 4], F32)
            nc.sync.dma_start(dcolT[:], dcol_d[:])
            yioT = cp.tile([128, SUMW], F32)
            nc.sync.dma_start(yioT[:], yio_d[:])
            wtT = cp.tile([128, NCH * 2 * OUT], BF16)
            nc.sync.dma_start(wtT[:], wt_d[:])
            biasT = cp.tile([OUT, 1], F32)
            nc.sync.dma_start(biasT[:], bias_d[:])
            hfracT = cp.tile([128, HPC], F32)
            nc.sync.dma_start(hfracT[:], hfrac_d[:])
            ident = cp.tile([128, 128], BF16)
            make_identity(nc, ident)
            partials = cp.tile([128, HPC * 2 * 128], BF16)

            s12pools = (tpsp, m5p, mbp, l1p, l2p)
            cvpools = (trp, cvp, ctp)

            def emit_chunk(hb, ch, hx5):
                vvs = CH_VVS[ch]
                nvv = len(vvs)
                nk = nvv * M * C
                sltiles = []
                nq = 0
                for vi in range(nvv):
                    slt = sp.tile([XP, M * 4 * YCMAX * C], BF16,
                                  name="slt", tag="slab")
                    fsz = M * 4 * YCMAX * C
                    step = fsz // DMA_SPLIT
                    for q in range(DMA_SPLIT):
                        lo = q * step
                        hi = fsz if q == DMA_SPLIT - 1 else (q + 1) * step
                        dmaq[nq % len(dmaq)].dma_start(
                            slt[:, lo:hi], slab_d[ch][hb * nvv + vi, :,
                                                      lo:hi])
                        nq += 1
                    sltiles.append(
                        slt[:].rearrange("p (u wb y c) -> p u wb y c",
                                         u=M, wb=4, y=YCMAX))
                for hl in range(HB):
                    h = hb * HB + hl
                    hy4 = _emit_yhats(nc, hyp, scr, dcolT, yioT, hfracT, ch, h, yoff)
                    cost = costp.tile([128, 4 * KCH], BF16,
                                      name="cost", tag="cost")
                    c3 = cost[:].rearrange("p (wb k) -> p wb k", wb=4)
                    if nk < KCH:
                        nc.gpsimd.memset(c3[:, :, nk:], 0.0)
                    xgT = xgp.tile([128, 4 * V], BF16, name="xgT", tag="xg")
                    nc.sync.dma_start(xgT[:], xg_d[h])
                    for ui in range(M):
                        _emit_stage12(nc, s12pools, sltiles, hx5, hy4, c3,
                                      ch, hl, ui)
                    # gate whole chunk
                    gv = xgT[:].rearrange("p (wb v) -> p wb v", wb=4)[
                        :, :, KOFF[ch] // C:KOFF[ch] // C + nvv * M] \
                        .unsqueeze(3).broadcast_to((128, 4, nvv * M, C))
                    ck = c3[:, :, :nk].rearrange("p wb (v c) -> p wb v c",
                                                 v=nvv * M)
                    nc.gpsimd.tensor_tensor(ck, ck, gv, OP.mult)
                    _emit_conv(nc, cvpools, c3, wtT, ident, partials, ch, h)

            for rep in range(reps):
                for hb in range(2):
                    hx5 = _emit_hx(nc, hxp, tbp, scr, tb_d, awxT, hb)
                    for ch in range(NCH):
                        emit_chunk(hb, ch, hx5)

            # epilogue: bias + output DMA
            ob = out_d[:].rearrange("o hh w n -> o hh (w n)")
            for h in range(HPC):
                for wb in range(4):
                    osb = outp.tile([OUT, 128], F32, name="osb", tag="osb")
                    nc.scalar.add(
                        osb[:],
                        partials[(wb % 2) * OUT:(wb % 2 + 1) * OUT,
                                 (h * 2 + wb // 2) * 128:
                                 (h * 2 + wb // 2 + 1) * 128],
                        biasT[:, 0:1])
                    nc.sync.dma_start(
                        ob[:, h, wb * 128:(wb + 1) * 128], osb[:])

    nc.compile()
    return nc


_CACHE = {}


def kernel(deltmap, imageMxM, x_g, conv_w, conv_b):
    deltmap = np.asarray(deltmap)
    imageMxM = np.asarray(imageMxM)
    x_g = np.asarray(x_g)
    conv_w = np.asarray(conv_w)
    conv_b = np.asarray(conv_b)
    assert np.abs(deltmap).max() <= 4.75, "offset outside window envelope"

    in_maps = [prep_core(k, deltmap, imageMxM, x_g, conv_w, conv_b)
               for k in range(NCORE)]

    if "nc" not in _CACHE:
        _CACHE["nc"] = build_nc()
    nc = _CACHE["nc"]

    res = bass_utils.run_bass_kernel_spmd(
        nc, in_maps, core_ids=list(range(NCORE)))
    outs = [res.results[k]["out"] for k in range(NCORE)]
    full = np.concatenate(outs, axis=1)
    return full[None].astype(np.float32)


# revision 3
# speedup vs baseline: 1.1687x; 1.1687x over previous
"""Trainium2 Bass kernel v2 for nn_BuildVolume.

Same math as v1 (dense-hat bilinear via TensorE x-contraction + per-sample
y-hat mult/reduce + gated 1x1 conv) restructured for engine balance:

- vv chunks grouped by |y-offset| in pairs: windows {44,32,24,12} (vs 39/29/13
  shared across 3 rows) and per-vv y-shifts baked into the slab by the host.
- x-hats built once per h-block (8 h) and cached, not rebuilt per chunk.
- stage-2 split across engines: Pool (gpsimd) multiplies straight out of
  PSUM (fused f32 read + bf16 cast), ACT casts for DVE bf16 multiplies,
  DVE runs a 2-level pairwise tree + short tensor_reduce instead of a full
  1x-rate reduce; gate applied in one pass per (chunk,h).
- conv transposes on the PE (identity matmul) instead of DMA transpose.

Sharding: H split 8 x 16, no collectives.
"""

import numpy as np
import ml_dtypes

import concourse.bacc as bacc
import concourse.mybir as mybir
from concourse.tile import TileContext
from concourse.masks import make_identity
from concourse import bass_utils

F32 = mybir.dt.float32
BF16 = mybir.dt.bfloat16
AX = mybir.AxisListType
OP = mybir.AluOpType
ACTF = mybir.ActivationFunctionType

B, H, W, N, C, OUT, M = 1, 128, 128, 4, 8, 64, 9
V = M * M
NCORE = 8
HPC = 16                     # rows per core
HB = 8                       # rows per h-block
PADX = 21
XP = 74
CH_VVS = [[0, 8], [1, 7], [2, 6], [3, 5, 4]]
WINS = [44, 32, 24, 16]
SHIFTS = [21, 16, 12, 7]
YCMAX = WINS[0] + HB - 1     # 51 (uniform slab tile y extent)
NCH = len(CH_VVS)
KCH = 256                    # padded k per chunk (2 x 128)
SUMW = sum(WINS)             # 112

# engine assignment per ui: mult path 'A' (ACT cast + DVE mult),
# 'P' (gpsimd direct from PSUM), 'D' (DVE direct from PSUM)
MULT_PATH = list('QQQQQAADD')
L1_ENG = list('DDPPPDDDD')
L2_ENG = list('DDDDDDDDD')
DMA_SPLIT = 4

KOFF = [0]
for _vvs in CH_VVS:
    KOFF.append(KOFF[-1] + len(_vvs) * M * C)


def _bf16(a):
    return np.asarray(a, dtype=ml_dtypes.bfloat16)


def prep_core(k, deltmap, imageMxM, x_g, conv_w, conv_b):
    """Host-side shard prep for core k (layout only)."""
    d = deltmap[0]
    g = x_g[0, 0]
    hg0 = HPC * k
    imgs = np.moveaxis(imageMxM[0], (3, 4), (0, 1)).reshape(V, H, W, C)
    s128 = np.arange(128)
    p = np.arange(XP)

    # slabs: per chunk one dram tensor [2*nvv, XP, M*4*YCMAX*C] bf16
    slabs = {}
    for ch, vvs in enumerate(CH_VVS):
        sh = SHIFTS[ch]
        nvv = len(vvs)
        sl = np.zeros((2 * nvv, XP, M, 4, YCMAX, C), np.float32)
        for hb in range(2):
            hb0 = hg0 + hb * HB
            y0 = hb0 - sh
            ys, ye = max(0, y0), min(H, y0 + YCMAX)
            for vi, vv in enumerate(vvs):
                for ui in range(M):
                    im = imgs[vv * M + ui]
                    for wb in range(4):
                        x0 = wb * 32 - PADX
                        xs, xe = max(0, x0), min(W, x0 + XP)
                        sl[hb * nvv + vi, xs - x0:xe - x0, ui, wb,
                           ys - y0:ye - y0, :] = \
                            im[ys:ye, xs:xe, :].transpose(1, 0, 2)
        slabs[f"slab{ch}"] = _bf16(sl.reshape(2 * nvv, XP, -1))

    # tb: [HPC, XP, 4*128] f32
    tb = np.zeros((HPC, XP, 4, 128), np.float32)
    for h in range(HPC):
        for wb in range(4):
            tb[h, :, wb, :] = d[hg0 + h, wb * 32 + s128 // 4, s128 % 4][None, :]

    # awx: [XP, 4*128] f32
    awx = np.zeros((XP, 4, 128), np.float32)
    for wb in range(4):
        wglob = wb * 32 + s128 // 4
        aw = wglob * (W / (W - 1.0)) - 0.5 - (wb * 32 - PADX)
        awx[:, wb, :] = aw[None, :] - p[:, None]

    # dcol: [128, HPC*4] f32
    dcol = np.zeros((128, HPC, 4), np.float32)
    for h in range(HPC):
        for wb in range(4):
            dcol[:, h, wb] = d[hg0 + h, wb * 32 + s128 // 4, s128 % 4]

    # yio: [128, SUMW] f32, h-independent (per-h frac offset folded into m0)
    yio = np.zeros((SUMW,), np.float32)
    off = 0
    for ch in range(NCH):
        win, sh = WINS[ch], SHIFTS[ch]
        yio[off:off + win] = np.arange(win) - sh + 0.5
        off += win
    yio = np.ascontiguousarray(np.broadcast_to(
        yio.reshape(1, SUMW), (128, SUMW)))

    # xg: [HPC, 128, 4*81] bf16, global k-order (ch, vi, ui)
    vord = [vv * M + uu for vvs in CH_VVS for vv in vvs for uu in range(M)]
    xg = np.zeros((HPC, 128, 4, V), np.float32)
    for h in range(HPC):
        for wb in range(4):
            xg[h, :, wb, :] = g[hg0 + h, wb * 32 + s128 // 4][:, vord]
    xg = _bf16(xg.reshape(HPC, 128, 4 * V))

    # wt: [128, NCH*2*OUT] bf16
    wt = np.zeros((128, NCH, 2, OUT), np.float32)
    for ch, vvs in enumerate(CH_VVS):
        nvv = len(vvs)
        for j in range(2):
            for pp in range(128):
                kk = j * 128 + pp
                if kk < nvv * M * C:
                    vi, r = divmod(kk, M * C)
                    uu, c = divmod(r, C)
                    wt[pp, ch, j, :] = conv_w[:, c * V + vvs[vi] * M + uu]
    wt = _bf16(wt.reshape(128, NCH * 2 * OUT))

    bias = conv_b.reshape(OUT, 1).astype(np.float32)

    hfrac = np.ascontiguousarray(np.broadcast_to(
        ((hg0 + np.arange(HPC)) / (H - 1.0)).astype(np.float32)
        .reshape(1, HPC), (128, HPC)))

    out = dict(tb=tb.reshape(HPC, XP, 512), awx=awx.reshape(XP, 512),
               dcol=dcol.reshape(128, HPC * 4), yio=yio, xg=xg, wt=wt,
               bias=bias, hfrac=hfrac)
    out.update(slabs)
    return out


def _emit_hx(nc, hxp, tbp, scr, tb_d, awxT, hb):
    """x-hats for one h-block: [XP, HB*M*4*128] bf16."""
    hx = hxp.tile([XP, HB * M * 512], BF16, name="hx", tag="hx")
    hx5 = hx[:].rearrange("p (h u s) -> p h u s", h=HB, u=M)
    for hl in range(HB):
        h = hb * HB + hl
        tbT = tbp.tile([XP, 512], F32, name="tbT", tag="tb")
        nc.sync.dma_start(tbT[:], tb_d[h])
        for ui in range(M):
            ic = float(M // 2 - ui)
            u = scr.tile([XP, 512], F32, name="u", tag="u")
            nc.vector.scalar_tensor_tensor(
                u[:], tbT[:], ic, awxT[:], OP.mult, OP.add)
            nc.scalar.activation(u[:], u[:], ACTF.Abs)
            nc.scalar.activation(hx5[:, hl, ui, :], u[:],
                                 ACTF.Relu, bias=1.0, scale=-1.0)
    return hx5


def _emit_yhats(nc, hyp, scr, dcolT, yioT, hfracT, ch, h, yoff):
    vvs = CH_VVS[ch]
    nvv = len(vvs)
    win = WINS[ch]
    hy = hyp.tile([128, 3 * 4 * WINS[0]], BF16, name="hy", tag="hy")
    hy4 = hy[:, :nvv * 4 * win].rearrange("p (v wb y) -> p v wb y",
                                          v=nvv, wb=4)
    for vi, vv in enumerate(vvs):
        bcoef = float(M // 2 - vv)
        m0 = scr.tile([128, 4], F32, name="m0", tag="m0")
        nc.vector.scalar_tensor_tensor(
            m0[:], dcolT[:, h * 4:(h + 1) * 4], bcoef,
            hfracT[:, h:h + 1].broadcast_to((128, 4)), OP.mult, OP.add)
        u4 = scr.tile([128, 4 * WINS[0]], F32, name="u4", tag="u4")
        u4v = u4[:, :4 * win].rearrange("p (wb y) -> p wb y", wb=4)
        ys = yoff[ch]
        nc.vector.tensor_tensor(
            u4v,
            yioT[:, ys:ys + win].unsqueeze(1).broadcast_to((128, 4, win)),
            m0[:].unsqueeze(2).broadcast_to((128, 4, win)),
            OP.subtract)
        nc.scalar.activation(u4[:, :4 * win], u4[:, :4 * win], ACTF.Abs)
        nc.scalar.activation(
            hy4[:, vi],
            u4[:, :4 * win].rearrange("p (wb y) -> p wb y", wb=4),
            ACTF.Relu, bias=1.0, scale=-1.0)
    return hy4


def _emit_stage12(nc, pools, sltiles, hx5, hy4, c3, ch, hl, ui):
    """stage-1 matmuls + stage-2 mult/tree/reduce for one (ch, hl, ui)."""
    tpsp, m5p, mbp, l1p, l2p = pools
    vvs = CH_VVS[ch]
    nvv = len(vvs)
    win = WINS[ch]
    nk = nvv * M * C
    cw = C * win
    nf = nvv * cw
    soff = 512 if nf > 512 else cw      # per-vi slot stride (bank-safe)
    lhs = hx5[:, hl, ui, :].rearrange("p (wb s) -> p wb s", wb=4)
    mp = MULT_PATH[ui]
    for wbp in range(2):
        m5 = m5p.tile([128, 2 * 704], BF16, name="m5", tag="m5")
        for wbi in range(2):
            wb = wbp * 2 + wbi
            tps = tpsp.tile([128, 1024], F32, name="tps", tag="tps")
            for vi in range(nvv):
                rhs = sltiles[vi][:, ui, wb, hl:hl + win, :] \
                    .rearrange("p y c -> p c y")
                nc.tensor.matmul(
                    tps[:, vi * soff:vi * soff + cw],
                    lhs[:, wb, :], rhs, start=True, stop=True)
            tview = tps[:, :nvv * soff].rearrange(
                "p (v r) -> p v r", v=nvv)[:, :, :cw].rearrange(
                "p v (c y) -> p v c y", c=C)
            hyv = hy4[:, :, wb, :].unsqueeze(2) \
                .broadcast_to((128, nvv, C, win))
            m5h = m5[:, wbi * nf:(wbi + 1) * nf].rearrange(
                "p (v c y) -> p v c y", v=nvv, c=C)
            if mp in ('A', 'Q'):
                mb = mbp.tile([128, 704], BF16, name="mb", tag="mbf")
                nc.scalar.copy(
                    mb[:, :nf].rearrange("p (v c y) -> p v c y",
                                         v=nvv, c=C), tview)
                eng = nc.vector if mp == 'A' else nc.gpsimd
                eng.tensor_tensor(
                    m5h, mb[:, :nf].rearrange("p (v c y) -> p v c y",
                                              v=nvv, c=C),
                    hyv, OP.mult)
            else:
                nc.vector.tensor_tensor(m5h, tview, hyv, OP.mult)
        # tree reduce: win -> win/2 -> win/4 -> 1
        g2 = 2 * nvv * C
        hw = win // 2
        qw = win // 4
        m3 = m5[:, :2 * nf].rearrange("p (g y) -> p g y", g=g2)
        l1 = l1p.tile([128, 2 * 352], BF16, name="l1", tag="l1")
        l1v = l1[:, :g2 * hw].rearrange("p (g y) -> p g y", g=g2)
        eng1 = nc.vector if L1_ENG[ui] == 'D' else nc.gpsimd
        eng1.tensor_tensor(l1v, m3[:, :, 0:hw], m3[:, :, hw:win], OP.add)
        l2 = l2p.tile([128, 2 * 176], BF16, name="l2", tag="l2")
        l2v = l2[:, :g2 * qw].rearrange("p (g y) -> p g y", g=g2)
        eng2 = nc.vector if L2_ENG[ui] == 'D' else nc.gpsimd
        eng2.tensor_tensor(l2v, l1v[:, :, 0:qw], l1v[:, :, qw:hw], OP.add)
        red_out = c3[:, wbp * 2:(wbp + 1) * 2, :nk].rearrange(
            "p w (v u c) -> p w v u c", v=nvv, u=M)[:, :, :, ui, :]
        # <=11 terms, almost all zero (hat support is 2 taps): bf16 safe
        with nc.allow_low_precision(reason="short window sum, 2 live taps"):
            nc.vector.tensor_reduce(
                red_out, l2v.rearrange("p (w v c) y -> p w v c y",
                                       w=2, v=nvv),
                AX.X, OP.add)


def _emit_conv(nc, pools, c3, wtT, ident, partials, ch, h):
    trp, cvp, ctp = pools
    for wb in range(4):
        cvt = cvp.tile([OUT, 128], F32, name="cvt", tag="cvt")
        for j in range(2):
            tr = trp.tile([128, 128], BF16, name="tr", tag="tr")
            nc.tensor.transpose(tr[:], c3[:, wb, j * 128:(j + 1) * 128],
                                ident[:])
            ct = ctp.tile([128, 128], BF16, name="ct", tag="ct")
            nc.vector.tensor_copy(ct[:], tr[:])
            nc.tensor.matmul(
                cvt[:],
                wtT[:, (ch * 2 + j) * OUT:(ch * 2 + j + 1) * OUT],
                ct[:], start=(j == 0), stop=(j == 1))
        ps = partials[(wb % 2) * OUT:(wb % 2 + 1) * OUT,
                      (h * 2 + wb // 2) * 128:(h * 2 + wb // 2 + 1) * 128]
        if ch == 0:
            nc.vector.tensor_copy(ps, cvt[:])
        else:
            nc.vector.tensor_tensor(ps, ps, cvt[:], OP.add)


def build_nc(reps=1):
    nc = bacc.Bacc("TRN2", target_bir_lowering=False)

    slab_d = []
    for ch, vvs in enumerate(CH_VVS):
        slab_d.append(nc.dram_tensor(
            f"slab{ch}", [2 * len(vvs), XP, M * 4 * YCMAX * C], BF16,
            kind="ExternalInput"))
    tb_d = nc.dram_tensor("tb", [HPC, XP, 512], F32, kind="ExternalInput")
    awx_d = nc.dram_tensor("awx", [XP, 512], F32, kind="ExternalInput")
    dcol_d = nc.dram_tensor("dcol", [128, HPC * 4], F32, kind="ExternalInput")
    yio_d = nc.dram_tensor("yio", [128, SUMW], F32, kind="ExternalInput")
    xg_d = nc.dram_tensor("xg", [HPC, 128, 4 * V], BF16, kind="ExternalInput")
    wt_d = nc.dram_tensor("wt", [128, NCH * 2 * OUT], BF16,
                          kind="ExternalInput")
    bias_d = nc.dram_tensor("bias", [OUT, 1], F32, kind="ExternalInput")
    hfrac_d = nc.dram_tensor("hfrac", [128, HPC], F32, kind="ExternalInput")
    out_d = nc.dram_tensor("out", [OUT, HPC, W, N], F32, kind="ExternalOutput")

    dmaq = [nc.sync, nc.scalar, nc.gpsimd]
    yoff = [0]
    for ch in range(NCH):
        yoff.append(yoff[-1] + WINS[ch])

    with TileContext(nc) as tc:
        with (
            tc.tile_pool(name="consts", bufs=1) as cp,
            tc.tile_pool(name="hxp", bufs=1) as hxp,
            tc.tile_pool(name="slabp", bufs=3) as sp,
            tc.tile_pool(name="tbp", bufs=2) as tbp,
            tc.tile_pool(name="scr", bufs=2) as scr,
            tc.tile_pool(name="hyp", bufs=2) as hyp,
            tc.tile_pool(name="m5p", bufs=3) as m5p,
            tc.tile_pool(name="mbp", bufs=2) as mbp,
            tc.tile_pool(name="l1p", bufs=2) as l1p,
            tc.tile_pool(name="l2p", bufs=2) as l2p,
            tc.tile_pool(name="costp", bufs=2) as costp,
            tc.tile_pool(name="xgp", bufs=2) as xgp,
            tc.tile_pool(name="ctp", bufs=2) as ctp,
            tc.tile_pool(name="outp", bufs=2) as outp,
            tc.tile_pool(name="tpsp", bufs=2, space="PSUM") as tpsp,
            tc.tile_pool(name="trp", bufs=1, space="PSUM") as trp,
            tc.tile_pool(name="cvp", bufs=1, space="PSUM") as cvp,
        ):
            awxT = cp.tile([XP, 512], F32)
            nc.sync.dma_start(awxT[:], awx_d[:])
            dcolT = cp.tile([128, HPC * 4], F32)
            nc.sync.dma_start(dcolT[:], dcol_d[:])
            yioT = cp.tile([128, SUMW], F32)
            nc.sync.dma_start(yioT[:], yio_d[:])
            wtT = cp.tile([128, NCH * 2 * OUT], BF16)
            nc.sync.dma_start(wtT[:], wt_d[:])
            biasT = cp.tile([OUT, 1], F32)
            nc.sync.dma_start(biasT[:], bias_d[:])
            hfracT = cp.tile([128, HPC], F32)
            nc.sync.dma_start(hfracT[:], hfrac_d[:])
            ident = cp.tile([128, 128], BF16)
            make_identity(nc, ident)
            partials = cp.tile([128, HPC * 2 * 128], BF16)

            s12pools = (tpsp, m5p, mbp, l1p, l2p)
            cvpools = (trp, cvp, ctp)

            def emit_chunk(hb, ch, hx5):
                vvs = CH_VVS[ch]
                nvv = len(vvs)
                nk = nvv * M * C
                sltiles = []
                nq = 0
                for vi in range(nvv):
                    slt = sp.tile([XP, M * 4 * YCMAX * C], BF16,
                                  name="slt", tag="slab")
                    fsz = M * 4 * YCMAX * C
                    step = fsz // DMA_SPLIT
                    for q in range(DMA_SPLIT):
                        lo = q * step
                        hi = fsz if q == DMA_SPLIT - 1 else (q + 1) * step
                        dmaq[nq % len(dmaq)].dma_start(
                            slt[:, lo:hi], slab_d[ch][hb * nvv + vi, :,
                                                      lo:hi])
                        nq += 1
                    sltiles.append(
                        slt[:].rearrange("p (u wb y c) -> p u wb y c",
                                         u=M, wb=4, y=YCMAX))
                for hl in range(HB):
                    h = hb * HB + hl
                    hy4 = _emit_yhats(nc, hyp, scr, dcolT, yioT, hfracT, ch, h, yoff)
                    cost = costp.tile([128, 4 * KCH], BF16,
                                      name="cost", tag="cost")
                    c3 = cost[:].rearrange("p (wb k) -> p wb k", wb=4)
                    if nk < KCH:
                        nc.gpsimd.memset(c3[:, :, nk:], 0.0)
                    xgT = xgp.tile([128, 4 * V], BF16, name="xgT", tag="xg")
                    nc.sync.dma_start(xgT[:], xg_d[h])
                    for ui in range(M):
                        _emit_stage12(nc, s12pools, sltiles, hx5, hy4, c3,
                                      ch, hl, ui)
                    # gate whole chunk
                    gv = xgT[:].rearrange("p (wb v) -> p wb v", wb=4)[
                        :, :, KOFF[ch] // C:KOFF[ch] // C + nvv * M] \
                        .unsqueeze(3).broadcast_to((128, 4, nvv * M, C))
                    ck = c3[:, :, :nk].rearrange("p wb (v c) -> p wb v c",
                                                 v=nvv * M)
                    nc.gpsimd.tensor_tensor(ck, ck, gv, OP.mult)
                    _emit_conv(nc, cvpools, c3, wtT, ident, partials, ch, h)

            for rep in range(reps):
                for hb in range(2):
                    hx5 = _emit_hx(nc, hxp, tbp, scr, tb_d, awxT, hb)
                    for ch in range(NCH):
                        emit_chunk(hb, ch, hx5)

            # epilogue: bias + output DMA
            ob = out_d[:].rearrange("o hh w n -> o hh (w n)")
            for h in range(HPC):
                for wb in range(4):
                    osb = outp.tile([OUT, 128], F32, name="osb", tag="osb")
                    nc.scalar.add(
                        osb[:],
                        partials[(wb % 2) * OUT:(wb % 2 + 1) * OUT,
                                 (h * 2 + wb // 2) * 128:
                                 (h * 2 + wb // 2 + 1) * 128],
                        biasT[:, 0:1])
                    nc.sync.dma_start(
                        ob[:, h, wb * 128:(wb + 1) * 128], osb[:])

    nc.compile()
    return nc


_CACHE = {}


def kernel(deltmap, imageMxM, x_g, conv_w, conv_b):
    deltmap = np.asarray(deltmap)
    imageMxM = np.asarray(imageMxM)
    x_g = np.asarray(x_g)
    conv_w = np.asarray(conv_w)
    conv_b = np.asarray(conv_b)
    assert np.abs(deltmap).max() <= 4.75, "offset outside window envelope"

    in_maps = [prep_core(k, deltmap, imageMxM, x_g, conv_w, conv_b)
               for k in range(NCORE)]

    if "nc" not in _CACHE:
        _CACHE["nc"] = build_nc()
    nc = _CACHE["nc"]

    res = bass_utils.run_bass_kernel_spmd(
        nc, in_maps, core_ids=list(range(NCORE)))
    outs = [res.results[k]["out"] for k in range(NCORE)]
    full = np.concatenate(outs, axis=1)
    return full[None].astype(np.float32)
